# revision 12
# baseline (speedup 1.0000x reference)
"""AMIP router kernel for 8 TRN2 NeuronCores.

Sharding: data-parallel over tokens. B=4 batches x M=1024 masked tokens
= 4096 tokens; core c handles 512 tokens (half of batch c//2's masked set).
Router/expert weights are replicated (streamed from each core's HBM).

Device math (all matmuls bf16 with f32 PSUM accumulation):
  phase A : h_avgT[d,t] = sum_s h_L[s,d] * AT[s,t]      (AT = averaging matrix,
            built on host from the index tensors; folds validity + 1/cnt)
  gate    : expT[k,t] = exp(W_r^T h_mask + b_r)          -- softmax denominator
            cancels in the final LayerNorm (scale invariance), so the gate is
            left unnormalized.
  phase B : H_T[h,t] = gelu(W1k^T X + b1k), Hg = H * g~  (X = [h_avg | h_mask])
  phase C : Y[t,d] = sum_k Hg_k^T W2k + exp^T b2, accumulated in PSUM,
            then LayerNorm over d and DMA out.

Host does only integer index prep, dtype casts, sharding and the final
scatter of LN rows into the zero output (tokens with no valid neighbors
keep zero rows).
"""

import numpy as np
import ml_dtypes

import concourse.bass as bass
import concourse.bacc as bacc
import concourse.tile as tile
import concourse.mybir as mybir
from concourse.bass_utils import run_bass_kernel_spmd

BF16 = mybir.dt.bfloat16
F32 = mybir.dt.float32
AF = mybir.ActivationFunctionType
ALU = mybir.AluOpType

B, S, D, K = 4, 2048, 4096, 8
M = S // 2
NCORES = 8
TOK = B * M // NCORES          # 512 tokens per core
DH = D // 4                    # 1024 expert hidden
TD = 2 * D                     # 8192 expert input
NT = TOK // 128                # 4 token chunks
NF = TD // 128                 # 64 X-feature chunks
ND = D // 128                  # 32 output-dim chunks
NH = DH // 128                 # 8 hidden chunks
NS = S // 128                  # 16 sequence chunks

_NC_CACHE = {}


def _build_nc():
    nc = bacc.Bacc("TRN2", target_bir_lowering=False, debug=False,
                   num_devices=NCORES)

    hL = nc.dram_tensor("hL", [S, D], BF16, kind="ExternalInput")
    hmT = nc.dram_tensor("hmT", [D, TOK], BF16, kind="ExternalInput")
    AT = nc.dram_tensor("AT", [S, TOK], BF16, kind="ExternalInput")
    W1 = nc.dram_tensor("W1", [K, TD, DH], BF16, kind="ExternalInput")
    W2 = nc.dram_tensor("W2", [K, DH, D], BF16, kind="ExternalInput")
    Wr = nc.dram_tensor("Wr", [128, ND * K], BF16, kind="ExternalInput")
    br = nc.dram_tensor("br", [K, 1], F32, kind="ExternalInput")
    b1 = nc.dram_tensor("b1", [128, K * NH], F32, kind="ExternalInput")
    b2 = nc.dram_tensor("b2", [K, D], BF16, kind="ExternalInput")
    sel = nc.dram_tensor("sel", [K, K * 128], BF16, kind="ExternalInput")
    out = nc.dram_tensor("out", [TOK, D], F32, kind="ExternalOutput")

    with tile.TileContext(nc) as tc:
        with (
            tc.tile_pool(name="hgt", bufs=K * NH) as p_hgt,
            tc.tile_pool(name="small", bufs=1) as p_small,
        ):
            # ---- small constants
            wr_sb = p_small.tile([128, ND * K], BF16)
            nc.sync.dma_start(wr_sb[:], Wr[:, :])
            br_sb = p_small.tile([K, 1], F32)
            nc.sync.dma_start(br_sb[:], br[:, :])
            b1_sb = p_small.tile([128, K * NH], F32)
            nc.sync.dma_start(b1_sb[:], b1[:, :])
            b2_sb = p_small.tile([K, D], BF16)
            nc.sync.dma_start(b2_sb[:], b2[:, :])
            expT = p_small.tile([K, TOK], BF16)
            eps_sb = p_small.tile([128, 1], F32)
            nc.gpsimd.memset(eps_sb[:], 1e-5)

            hgt = [None] * (K * NH)

            with tc.tile_pool(name="xt", bufs=NF) as p_xt:
                # ---- XT lower half: h_maskT straight from DRAM
                xt = [None] * NF
                for f in range(ND, NF):
                    t = p_xt.tile([128, TOK], BF16)
                    nc.sync.dma_start(
                        t[:], hmT[(f - ND) * 128:(f - ND + 1) * 128, :])
                    xt[f] = t

                # ---- gate: logitsT[k,t] accumulated over D, then exp
                with tc.tile_pool(name="psG", bufs=1, space="PSUM") as psG:
                    ps_g = psG.tile([K, TOK], F32)
                    for i in range(ND):
                        nc.tensor.matmul(
                            ps_g[:], wr_sb[:, i * K:(i + 1) * K], xt[ND + i][:],
                            start=(i == 0), stop=(i == ND - 1))
                    nc.scalar.activation(expT[:], ps_g[:], AF.Exp,
                                         bias=br_sb[:, 0:1])

                # ---- broadcast each gate row to all 128 partitions via a
                # one-hot selector matmul (gb_all[k][p, t] = expT[k, t])
                gb_all = []
                sel_sb = p_small.tile([K, K * 128], BF16)
                nc.sync.dma_start(sel_sb[:], sel[:, :])
                with tc.tile_pool(name="psGB", bufs=2, space="PSUM") as psGB:
                    for k in range(K):
                        pg = psGB.tile([128, TOK], F32, name="pgb", tag="pgb")
                        nc.tensor.matmul(pg[:], sel_sb[:, k * 128:(k + 1) * 128],
                                         expT[:], start=True, stop=True)
                        gb = p_hgt.tile([128, TOK], BF16, name="gball",
                                        tag="gball", bufs=K)
                        nc.scalar.copy(gb[:], pg[:])
                        gb_all.append(gb)

                # ---- phase A: h_avgT = hL^T @ AT, into XT upper half
                with (
                    tc.tile_pool(name="at", bufs=NS) as p_at,
                    tc.tile_pool(name="hls", bufs=3) as p_hl,
                    tc.tile_pool(name="psA", bufs=8, space="PSUM") as psA,
                ):
                    at = []
                    for s in range(NS):
                        t = p_at.tile([128, TOK], BF16)
                        nc.sync.dma_start(t[:], AT[s * 128:(s + 1) * 128, :])
                        at.append(t)
                    for dg in range(4):           # 8 d-chunks per group
                        pst = [psA.tile([128, TOK], F32, name="psa", tag="psa") for _ in range(8)]
                        for s in range(NS):
                            slab = p_hl.tile([128, 1024], BF16)
                            nc.sync.dma_start(
                                slab[:], hL[s * 128:(s + 1) * 128,
                                            dg * 1024:(dg + 1) * 1024])
                            for j in range(8):
                                nc.tensor.matmul(
                                    pst[j][:], slab[:, j * 128:(j + 1) * 128],
                                    at[s][:], start=(s == 0), stop=(s == NS - 1))
                        for j in range(8):
                            t = p_xt.tile([128, TOK], BF16)
                            nc.scalar.copy(t[:], pst[j][:])
                            xt[dg * 8 + j] = t

                # ---- phase B: per-expert hidden, gelu, gate multiply
                with (
                    tc.tile_pool(name="w1s", bufs=4) as p_w1,
                    tc.tile_pool(name="htmp", bufs=4) as p_h,
                    tc.tile_pool(name="psB", bufs=8, space="PSUM") as psB,
                ):
                    for k in range(K):
                        pst = [psB.tile([128, TOK], F32, name="psb", tag="psb") for _ in range(NH)]
                        for f in range(NF):
                            slab = p_w1.tile([128, DH], BF16)
                            nc.sync.dma_start(
                                slab[:], W1[k, f * 128:(f + 1) * 128, :])
                            for h in range(NH):
                                nc.tensor.matmul(
                                    pst[h][:], slab[:, h * 128:(h + 1) * 128],
                                    xt[f][:], start=(f == 0), stop=(f == NF - 1))
                        for h in range(NH):
                            ht = p_h.tile([128, TOK], BF16)
                            col = k * NH + h
                            nc.scalar.activation(ht[:], pst[h][:], AF.Gelu,
                                                 bias=b1_sb[:, col:col + 1])
                            g = p_hgt.tile([128, TOK], BF16)
                            nc.vector.tensor_mul(g[:], ht[:], gb_all[k][:])
                            hgt[col] = g
            # p_xt released: phase C's Y tiles reuse its address space.

            # ---- phase C: Y = sum_k Hg_k^T @ W2k + expT^T @ b2; LayerNorm
            with (
                tc.tile_pool(name="w2s", bufs=6) as p_w2,
                tc.tile_pool(name="ysb", bufs=NT) as p_y,
                tc.tile_pool(name="sq", bufs=2) as p_sq,
                tc.tile_pool(name="stat", bufs=16) as p_stat,
                tc.tile_pool(name="psC", bufs=8, space="PSUM") as psC,
            ):
                ysb = [p_y.tile([128, D], F32, name="ysb", tag="ysb") for _ in range(NT)]
                sums = [p_stat.tile([128, ND // 4], F32, name="sums", tag="sums") for _ in range(NT)]
                sumsq = [p_stat.tile([128, ND // 4], F32, name="sumsq", tag="sumsq") for _ in range(NT)]
                for dcp in range(4):          # pairs of 512-wide d column blocks
                    pst = [psC.tile([128, 512], F32, name="psc", tag="psc") for _ in range(8)]
                    for k in range(K):
                        for h in range(NH):
                            slab = p_w2.tile([128, 1024], BF16)
                            nc.sync.dma_start(
                                slab[:], W2[k, h * 128:(h + 1) * 128,
                                            dcp * 1024:(dcp + 1) * 1024])
                            first = (k == 0 and h == 0)
                            for half in range(2):
                                for t in range(NT):
                                    nc.tensor.matmul(
                                        pst[half * NT + t][:],
                                        hgt[k * NH + h][:, t * 128:(t + 1) * 128],
                                        slab[:, half * 512:(half + 1) * 512],
                                        start=first, stop=False)
                    for half in range(2):
                        dc = dcp * 2 + half
                        for t in range(NT):
                            nc.tensor.matmul(
                                pst[half * NT + t][:],
                                expT[:, t * 128:(t + 1) * 128],
                                b2_sb[:, dc * 512:(dc + 1) * 512],
                                start=False, stop=True)
                    for half in range(2):
                        dc = dcp * 2 + half
                        for t in range(NT):
                            p = pst[half * NT + t]
                            nc.scalar.activation(
                                ysb[t][:, dc * 512:(dc + 1) * 512], p[:],
                                AF.Identity, accum_out=sums[t][:, dc:dc + 1])
                            sq = p_sq.tile([128, 512], F32)
                            nc.scalar.activation(
                                sq[:], p[:], AF.Square,
                                accum_out=sumsq[t][:, dc:dc + 1])

                # ---- LayerNorm rows (mean/var over d) and output
                inv_d = 1.0 / D
                for t in range(NT):
                    s1 = p_stat.tile([128, 1], F32)
                    nc.vector.tensor_reduce(s1[:], sums[t][:, :],
                                            mybir.AxisListType.X, ALU.add)
                    s2 = p_stat.tile([128, 1], F32)
                    nc.vector.tensor_reduce(s2[:], sumsq[t][:, :],
                                            mybir.AxisListType.X, ALU.add)
                    mu = p_stat.tile([128, 1], F32)
                    nc.vector.tensor_scalar_mul(mu[:], s1[:], inv_d)
                    ex2 = p_stat.tile([128, 1], F32)
                    nc.vector.tensor_scalar_mul(ex2[:], s2[:], inv_d)
                    musq = p_stat.tile([128, 1], F32)
                    nc.vector.tensor_mul(musq[:], mu[:], mu[:])
                    var = p_stat.tile([128, 1], F32)
                    nc.vector.tensor_sub(var[:], ex2[:], musq[:])
                    std = p_stat.tile([128, 1], F32)
                    nc.scalar.activation(std[:], var[:], AF.Sqrt,
                                         bias=eps_sb[:, 0:1])
                    rstd = p_stat.tile([128, 1], F32)
                    nc.vector.reciprocal(rstd[:], std[:])
                    nc.vector.tensor_scalar(ysb[t][:], ysb[t][:], mu[:],
                                            rstd[:], ALU.subtract, ALU.mult)
                    nc.sync.dma_start(out[t * 128:(t + 1) * 128, :], ysb[t][:])

    nc.compile()
    return nc


def get_nc():
    if "nc" not in _NC_CACHE:
        _NC_CACHE["nc"] = _build_nc()
    return _NC_CACHE["nc"]


def _host_prep(h_L, W_r, b_r, W1, b1, W2, b2, mask_indices, unmasked_indices,
               range_r):
    """Integer-index prep + dtype casts + sharding. Returns in_maps and
    the scatter plan for unsharding."""
    r = int(range_r)
    bf = ml_dtypes.bfloat16

    is_un = np.zeros((B, S), bool)
    is_un[np.arange(B)[:, None], unmasked_indices] = True
    if r > 0:
        offs = np.concatenate([np.arange(-r, 0), np.arange(1, r + 1)])
        pos = mask_indices[:, :, None] + offs[None, None, :]      # [B,M,2r]
        inb = (pos >= 0) & (pos < S)
        posc = np.clip(pos, 0, S - 1)
        valid = inb & is_un[np.arange(B)[:, None, None], posc]
        cnt = valid.sum(-1)
    else:
        cnt = np.zeros((B, M), np.int64)

    ATf = np.zeros((B, S, M), np.float32)
    if r > 0:
        w = (1.0 / np.maximum(cnt, 1)).astype(np.float32)
        b_ix, m_ix, j_ix = np.nonzero(valid)
        ATf[b_ix, posc[b_ix, m_ix, j_ix], m_ix] = w[b_ix, m_ix]

    W1b = np.ascontiguousarray(W1).astype(bf)
    W2b = np.ascontiguousarray(W2).astype(bf)
    b2b = np.ascontiguousarray(b2).astype(bf)
    # Wr rearranged so chunk dc lives in columns [dc*K, (dc+1)*K)
    Wrb = np.ascontiguousarray(
        W_r.reshape(ND, 128, K).transpose(1, 0, 2).reshape(128, ND * K)
    ).astype(bf)
    brf = np.ascontiguousarray(b_r.reshape(K, 1)).astype(np.float32)
    # b1 col k*NH+h = b1[k, h*128:(h+1)*128]
    b1f = np.ascontiguousarray(
        b1.reshape(K, NH, 128).transpose(2, 0, 1).reshape(128, K * NH)
    ).astype(np.float32)

    hLb = [np.ascontiguousarray(h_L[b]).astype(bf) for b in range(B)]

    selb = np.zeros((K, K * 128), bf)
    for k in range(K):
        selb[k, k * 128:(k + 1) * 128] = 1

    in_maps = []
    plans = []
    per_batch = M // (NCORES // B)            # 512
    for c in range(NCORES):
        b = c // (NCORES // B)
        t0 = (c % (NCORES // B)) * per_batch
        toks = mask_indices[b, t0:t0 + per_batch]
        hmT = np.ascontiguousarray(h_L[b][toks].T).astype(bf)
        ATc = np.ascontiguousarray(ATf[b][:, t0:t0 + per_batch]).astype(bf)
        in_maps.append({
            "hL": hLb[b], "hmT": hmT, "AT": ATc,
            "W1": W1b, "W2": W2b, "Wr": Wrb, "br": brf, "b1": b1f, "b2": b2b,
            "sel": selb,
        })
        plans.append((b, toks, cnt[b, t0:t0 + per_batch] > 0))
    return in_maps, plans


def kernel(h_L, W_r, b_r, W1, b1, W2, b2, mask_indices, unmasked_indices,
           range_r):
    h_L = np.asarray(h_L, np.float32)
    mask_indices = np.asarray(mask_indices)
    unmasked_indices = np.asarray(unmasked_indices)
    assert h_L.shape == (B, S, D) and mask_indices.shape == (B, M)

    in_maps, plans = _host_prep(
        h_L, np.asarray(W_r, np.float32), np.asarray(b_r, np.float32),
        np.asarray(W1, np.float32), np.asarray(b1, np.float32),
        np.asarray(W2, np.float32), np.asarray(b2, np.float32),
        mask_indices, unmasked_indices, range_r)

    nc = get_nc()
    res = run_bass_kernel_spmd(nc, in_maps, core_ids=list(range(NCORES)))

    full = np.zeros((B, S, D), np.float32)
    for c in range(NCORES):
        b, toks, has = plans[c]
        o = np.asarray(res.results[c]["out"], np.float32)
        full[b, toks[has]] = o[has]
    return full


# revision 21
# speedup vs baseline: 2.9523x; 2.9523x over previous
"""AMIP router kernel for 8 TRN2 NeuronCores.

Sharding: data-parallel over tokens. B=4 batches x M=1024 masked tokens
= 4096 tokens; core c handles 512 tokens (half of batch c//2's masked set).
Router/expert weights are replicated (streamed from each core's HBM).

Device math (all matmuls bf16 with f32 PSUM accumulation):
  phase A : h_avgT[d,t] = sum_s hLw[s,d] * ATw[s,t] per 128-token chunk,
            where hLw/ATw are per-chunk windows of h_L and the averaging
            matrix (tokens are sorted, so a 128-token chunk only touches a
            ~WIN-row band of the sequence; host shifts each window to start
            at row 0 so the SPMD graph stays static).
  gate    : expT[k,t] = exp(W_r^T h_mask + b_r)          -- softmax denominator
            cancels in the final LayerNorm (scale invariance), so the gate is
            left unnormalized.
  phase B : H_T[h,t] = gelu(W1k^T X + b1k), Hg = H * g~  (X = [h_avg | h_mask])
  phase C : Y[t,d] = sum_k Hg_k^T W2k + exp^T b2, accumulated in PSUM,
            then LayerNorm over d and DMA out.

Host does only integer index prep, dtype casts, sharding and the final
scatter of LN rows into the zero output (tokens with no valid neighbors
keep zero rows).
"""

import os
import numpy as np
import ml_dtypes

import concourse.bass as bass
import concourse.bacc as bacc
import concourse.tile as tile
import concourse.mybir as mybir
from concourse.bass_utils import run_bass_kernel_spmd

BF16 = mybir.dt.bfloat16
F32 = mybir.dt.float32
AF = mybir.ActivationFunctionType
ALU = mybir.AluOpType

B, S, D, K = 4, 2048, 4096, 8
M = S // 2
NCORES = 8
TOK = B * M // NCORES          # 512 tokens per core
DH = D // 4                    # 1024 expert hidden
TD = 2 * D                     # 8192 expert input
NT = TOK // 128                # 4 token chunks
NF = TD // 128                 # 64 X-feature chunks
ND = D // 128                  # 32 output-dim chunks
NH = DH // 128                 # 8 hidden chunks

_NC_CACHE = {}


def _build_nc(win):
    nw = win // 128
    nc = bacc.Bacc("TRN2", target_bir_lowering=False, debug=False,
                   num_devices=NCORES)

    hLw = nc.dram_tensor("hLw", [NT, win, D], BF16, kind="ExternalInput")
    ATw = nc.dram_tensor("ATw", [NT, win, 128], BF16, kind="ExternalInput")
    hmT = nc.dram_tensor("hmT", [D, TOK], BF16, kind="ExternalInput")
    W1 = nc.dram_tensor("W1", [K, TD, DH], BF16, kind="ExternalInput")
    W2 = nc.dram_tensor("W2", [K, DH, D], BF16, kind="ExternalInput")
    Wr = nc.dram_tensor("Wr", [128, ND * K], BF16, kind="ExternalInput")
    br = nc.dram_tensor("br", [K, 1], F32, kind="ExternalInput")
    b1 = nc.dram_tensor("b1", [128, K * NH], F32, kind="ExternalInput")
    b2 = nc.dram_tensor("b2", [K, D], BF16, kind="ExternalInput")
    sel = nc.dram_tensor("sel", [K, K * 128], BF16, kind="ExternalInput")
    out = nc.dram_tensor("out", [TOK, D], F32, kind="ExternalOutput")

    with tile.TileContext(nc) as tc:
        with (
            tc.tile_pool(name="hgt", bufs=K * NH) as p_hgt,
            tc.tile_pool(name="small", bufs=1) as p_small,
        ):
            # ---- small constants
            wr_sb = p_small.tile([128, ND * K], BF16)
            nc.sync.dma_start(wr_sb[:], Wr[:, :])
            br_sb = p_small.tile([K, 1], F32)
            nc.sync.dma_start(br_sb[:], br[:, :])
            b1_sb = p_small.tile([128, K * NH], F32)
            nc.sync.dma_start(b1_sb[:], b1[:, :])
            b2_sb = p_small.tile([K, D], BF16)
            nc.sync.dma_start(b2_sb[:], b2[:, :])
            sel_sb = p_small.tile([K, K * 128], BF16)
            nc.sync.dma_start(sel_sb[:], sel[:, :])
            expT = p_small.tile([K, TOK], BF16)
            eps_sb = p_small.tile([128, 1], F32)
            nc.gpsimd.memset(eps_sb[:], 1e-5)

            hgt = [None] * (K * NH)

            with tc.tile_pool(name="xt", bufs=NF) as p_xt:
                # ---- XT lower half: h_maskT straight from DRAM
                xt = [None] * NF
                for f in range(ND, NF):
                    t = p_xt.tile([128, TOK], BF16)
                    nc.sync.dma_start(
                        t[:], hmT[(f - ND) * 128:(f - ND + 1) * 128, :])
                    xt[f] = t

                # ---- phase A: windowed h_avgT into XT upper half.
                # One PSUM bank holds all 4 token-quarters of a d-chunk
                # (independent accumulation groups per 128-col slice), so
                # each d-chunk drains with a single ACT copy.
                pha_groups = os.environ.get("AMIP_PHA", "grp") == "grp"
                with (
                    tc.tile_pool(name="atw", bufs=NT * nw) as p_at,
                    tc.tile_pool(name="hlw", bufs=4) as p_hl,
                    tc.tile_pool(name="psA", bufs=8, space="PSUM") as psA,
                ):
                    atw = []
                    for i in range(NT):
                        for s in range(nw):
                            t = p_at.tile([128, 128], BF16, name="atw",
                                          tag="atw")
                            nc.sync.dma_start(
                                t[:], ATw[i, s * 128:(s + 1) * 128, :])
                            atw.append(t)
                    if pha_groups:
                        for dcg in range(ND // 4):    # groups of 4 d-chunks
                            pts = [psA.tile([128, TOK], F32, name="psa",
                                            tag="psa") for _ in range(4)]
                            for i in range(NT):
                                for s in range(nw):
                                    slab = p_hl.tile([128, 512], BF16)
                                    nc.sync.dma_start(
                                        slab[:], hLw[i, s * 128:(s + 1) * 128,
                                                     dcg * 512:(dcg + 1) * 512])
                                    for j in range(4):
                                        nc.tensor.matmul(
                                            pts[j][:, i * 128:(i + 1) * 128],
                                            slab[:, j * 128:(j + 1) * 128],
                                            atw[i * nw + s][:],
                                            start=(s == 0), stop=(s == nw - 1))
                            for j in range(4):
                                t = p_xt.tile([128, TOK], BF16)
                                nc.scalar.copy(t[:], pts[j][:])
                                xt[dcg * 4 + j] = t
                    else:
                        # one [128,128] PSUM tile per (d-chunk, token-chunk);
                        # drains split between ACT and DVE
                        for dcg in range(ND // 2):    # groups of 2 d-chunks
                            pts = [psA.tile([128, 128], F32, name="psa2",
                                            tag="psa2") for _ in range(8)]
                            for i in range(NT):
                                for s in range(nw):
                                    slab = p_hl.tile([128, 256], BF16,
                                                     name="slab2", tag="slab2")
                                    nc.sync.dma_start(
                                        slab[:], hLw[i, s * 128:(s + 1) * 128,
                                                     dcg * 256:(dcg + 1) * 256])
                                    for j in range(2):
                                        nc.tensor.matmul(
                                            pts[j * NT + i][:],
                                            slab[:, j * 128:(j + 1) * 128],
                                            atw[i * nw + s][:],
                                            start=(s == 0), stop=(s == nw - 1))
                            for j in range(2):
                                t = p_xt.tile([128, TOK], BF16)
                                for i in range(NT):
                                    src = pts[j * NT + i]
                                    dst = t[:, i * 128:(i + 1) * 128]
                                    if i % 2 == 0:
                                        nc.scalar.copy(dst, src[:])
                                    else:
                                        nc.vector.tensor_copy(dst, src[:])
                                xt[dcg * 2 + j] = t

                # ---- gate: logitsT[k,t] accumulated over D, then exp
                with tc.tile_pool(name="psG", bufs=1, space="PSUM") as psG:
                    ps_g = psG.tile([K, TOK], F32)
                    for i in range(ND):
                        nc.tensor.matmul(
                            ps_g[:], wr_sb[:, i * K:(i + 1) * K], xt[ND + i][:],
                            start=(i == 0), stop=(i == ND - 1))
                    nc.scalar.activation(expT[:], ps_g[:], AF.Exp,
                                         bias=br_sb[:, 0:1])

                # ---- broadcast each gate row to all 128 partitions via a
                # one-hot selector matmul (gb_all[k][p, t] = expT[k, t])
                gb_all = []
                with tc.tile_pool(name="psGB", bufs=2, space="PSUM") as psGB:
                    for k in range(K):
                        pg = psGB.tile([128, TOK], F32, name="pgb", tag="pgb")
                        nc.tensor.matmul(pg[:], sel_sb[:, k * 128:(k + 1) * 128],
                                         expT[:], start=True, stop=True)
                        gb = p_hgt.tile([128, TOK], BF16, name="gball",
                                        tag="gball", bufs=K)
                        nc.scalar.copy(gb[:], pg[:])
                        gb_all.append(gb)

                # ---- phase B: per-expert hidden, gelu, gate multiply
                with (
                    tc.tile_pool(name="w1s", bufs=4) as p_w1,
                    tc.tile_pool(name="htmp", bufs=4) as p_h,
                    tc.tile_pool(name="psB", bufs=8, space="PSUM") as psB,
                ):
                    for k in range(K):
                        pst = [psB.tile([128, TOK], F32, name="psb",
                                        tag="psb") for _ in range(NH)]
                        for f in range(NF):
                            slab = p_w1.tile([128, DH], BF16)
                            nc.sync.dma_start(
                                slab[:], W1[k, f * 128:(f + 1) * 128, :])
                            for h in range(NH):
                                nc.tensor.matmul(
                                    pst[h][:], slab[:, h * 128:(h + 1) * 128],
                                    xt[f][:], start=(f == 0), stop=(f == NF - 1))
                        act_fn = (AF.Relu if os.environ.get("AMIP_ACT") == "relu"
                                  else AF.Gelu)
                        for h in range(NH):
                            ht = p_h.tile([128, TOK], BF16)
                            col = k * NH + h
                            nc.scalar.activation(ht[:], pst[h][:], act_fn,
                                                 bias=b1_sb[:, col:col + 1])
                            g = p_hgt.tile([128, TOK], BF16)
                            nc.vector.tensor_mul(g[:], ht[:], gb_all[k][:])
                            hgt[col] = g
            # p_xt released: phase C's Y tiles reuse its address space.

            # ---- phase C: Y = sum_k Hg_k^T @ W2k + expT^T @ b2; LayerNorm
            with (
                tc.tile_pool(name="w2s", bufs=6) as p_w2,
                tc.tile_pool(name="ysb", bufs=NT) as p_y,
                tc.tile_pool(name="sq", bufs=2) as p_sq,
                tc.tile_pool(name="stat", bufs=16) as p_stat,
                tc.tile_pool(name="psC", bufs=8, space="PSUM") as psC,
            ):
                ysb = [p_y.tile([128, D], F32, name="ysb", tag="ysb")
                       for _ in range(NT)]
                sums = [p_stat.tile([128, 8], F32, name="sums", tag="sums")
                        for _ in range(NT)]
                sumsq = [p_stat.tile([128, 8], F32, name="sumsq", tag="sumsq")
                         for _ in range(NT)]
                for dc in range(8):               # 512-wide d column blocks
                    pst = [psC.tile([128, 512], F32, name="psc", tag="psc")
                           for _ in range(NT)]
                    for k in range(K):
                        for h in range(NH):
                            slab = p_w2.tile([128, 512], BF16)
                            nc.sync.dma_start(
                                slab[:], W2[k, h * 128:(h + 1) * 128,
                                            dc * 512:(dc + 1) * 512])
                            first = (k == 0 and h == 0)
                            for t in range(NT):
                                nc.tensor.matmul(
                                    pst[t][:],
                                    hgt[k * NH + h][:, t * 128:(t + 1) * 128],
                                    slab[:], start=first, stop=False)
                    for t in range(NT):
                        nc.tensor.matmul(
                            pst[t][:], expT[:, t * 128:(t + 1) * 128],
                            b2_sb[:, dc * 512:(dc + 1) * 512],
                            start=False, stop=True)
                    for t in range(NT):
                        p = pst[t]
                        nc.scalar.activation(
                            ysb[t][:, dc * 512:(dc + 1) * 512], p[:],
                            AF.Identity, accum_out=sums[t][:, dc:dc + 1])
                        sq = p_sq.tile([128, 512], F32)
                        nc.scalar.activation(
                            sq[:], p[:], AF.Square,
                            accum_out=sumsq[t][:, dc:dc + 1])

                # ---- LayerNorm rows (mean/var over d) and output.
                # Apply is split: ACT does the left half, DVE the right half.
                skip_ln = os.environ.get("AMIP_ABLATE") == "noln"
                inv_d = 1.0 / D
                half = D // 2
                for t in range(NT):
                    if skip_ln:
                        nc.sync.dma_start(out[t * 128:(t + 1) * 128, :],
                                          ysb[t][:])
                        continue
                    s1 = p_stat.tile([128, 1], F32)
                    nc.vector.tensor_reduce(s1[:], sums[t][:, :],
                                            mybir.AxisListType.X, ALU.add)
                    s2 = p_stat.tile([128, 1], F32)
                    nc.vector.tensor_reduce(s2[:], sumsq[t][:, :],
                                            mybir.AxisListType.X, ALU.add)
                    mu = p_stat.tile([128, 1], F32)
                    nc.vector.tensor_scalar_mul(mu[:], s1[:], inv_d)
                    ex2 = p_stat.tile([128, 1], F32)
                    nc.vector.tensor_scalar_mul(ex2[:], s2[:], inv_d)
                    musq = p_stat.tile([128, 1], F32)
                    nc.vector.tensor_mul(musq[:], mu[:], mu[:])
                    var = p_stat.tile([128, 1], F32)
                    nc.vector.tensor_sub(var[:], ex2[:], musq[:])
                    std = p_stat.tile([128, 1], F32)
                    nc.scalar.activation(std[:], var[:], AF.Sqrt,
                                         bias=eps_sb[:, 0:1])
                    rstd = p_stat.tile([128, 1], F32)
                    nc.vector.reciprocal(rstd[:], std[:])
                    nmr = p_stat.tile([128, 1], F32)
                    nc.vector.tensor_mul(nmr[:], mu[:], rstd[:])
                    nc.vector.tensor_scalar_mul(nmr[:], nmr[:], -1.0)
                    # left half on ACT: y*rstd - mu*rstd
                    nc.scalar.activation(ysb[t][:, :half], ysb[t][:, :half],
                                         AF.Identity, bias=nmr[:, 0:1],
                                         scale=rstd[:, 0:1])
                    nc.sync.dma_start(out[t * 128:(t + 1) * 128, :half],
                                      ysb[t][:, :half])
                    # right half on DVE: (y - mu) * rstd
                    nc.vector.tensor_scalar(ysb[t][:, half:], ysb[t][:, half:],
                                            mu[:], rstd[:],
                                            ALU.subtract, ALU.mult)
                    nc.sync.dma_start(out[t * 128:(t + 1) * 128, half:],
                                      ysb[t][:, half:])

    nc.compile()
    return nc


def get_nc(win=384):
    if win not in _NC_CACHE:
        _NC_CACHE[win] = _build_nc(win)
    return _NC_CACHE[win]


def _host_prep(h_L, W_r, b_r, W1, b1, W2, b2, mask_indices, unmasked_indices,
               range_r):
    """Integer-index prep + dtype casts + sharding. Returns (win, in_maps,
    scatter plans)."""
    r = int(range_r)
    bf = ml_dtypes.bfloat16

    is_un = np.zeros((B, S), bool)
    is_un[np.arange(B)[:, None], unmasked_indices] = True
    if r > 0:
        offs = np.concatenate([np.arange(-r, 0), np.arange(1, r + 1)])
        pos = mask_indices[:, :, None] + offs[None, None, :]      # [B,M,2r]
        inb = (pos >= 0) & (pos < S)
        posc = np.clip(pos, 0, S - 1)
        valid = inb & is_un[np.arange(B)[:, None, None], posc]
        cnt = valid.sum(-1)
        w = (1.0 / np.maximum(cnt, 1)).astype(np.float32)
    else:
        cnt = np.zeros((B, M), np.int64)

    W1b = np.ascontiguousarray(W1).astype(bf)
    W2b = np.ascontiguousarray(W2).astype(bf)
    b2b = np.ascontiguousarray(b2).astype(bf)
    # Wr rearranged so chunk dc lives in columns [dc*K, (dc+1)*K)
    Wrb = np.ascontiguousarray(
        W_r.reshape(ND, 128, K).transpose(1, 0, 2).reshape(128, ND * K)
    ).astype(bf)
    brf = np.ascontiguousarray(b_r.reshape(K, 1)).astype(np.float32)
    # b1 col k*NH+h = b1[k, h*128:(h+1)*128]
    b1f = np.ascontiguousarray(
        b1.reshape(K, NH, 128).transpose(2, 0, 1).reshape(128, K * NH)
    ).astype(np.float32)

    hLb = [np.ascontiguousarray(h_L[b]).astype(bf) for b in range(B)]

    selb = np.zeros((K, K * 128), bf)
    for k in range(K):
        selb[k, k * 128:(k + 1) * 128] = 1

    per_batch = M // (NCORES // B)            # 512 tokens per core
    # window size: max span of any 128-token chunk's neighbor band
    win = 256
    for b in range(B):
        for c in range(NCORES // B):
            toks = mask_indices[b, c * per_batch:(c + 1) * per_batch]
            for i in range(NT):
                ch = toks[i * 128:(i + 1) * 128]
                span = int(ch[-1]) + r - (int(ch[0]) - r) + 1
                win = max(win, -(-span // 128) * 128)
    win = min(win, -(-S // 128) * 128)

    in_maps = []
    plans = []
    for c in range(NCORES):
        b = c // (NCORES // B)
        t0 = (c % (NCORES // B)) * per_batch
        toks = mask_indices[b, t0:t0 + per_batch]
        hmTc = np.ascontiguousarray(h_L[b][toks].T).astype(bf)
        hLwc = np.zeros((NT, win, D), bf)
        ATwc = np.zeros((NT, win, 128), np.float32)
        for i in range(NT):
            ch = toks[i * 128:(i + 1) * 128]
            w0 = min(max(int(ch[0]) - r, 0), S - win)
            hLwc[i] = hLb[b][w0:w0 + win]
            if r > 0:
                mrow = t0 + i * 128
                v = valid[b, mrow:mrow + 128]            # [128, 2r]
                pc = posc[b, mrow:mrow + 128]
                jj, oo = np.nonzero(v)
                ATwc[i, pc[jj, oo] - w0, jj] = w[b, mrow + jj]
        in_maps.append({
            "hLw": hLwc, "ATw": ATwc.astype(bf), "hmT": hmTc,
            "W1": W1b, "W2": W2b, "Wr": Wrb, "br": brf, "b1": b1f, "b2": b2b,
            "sel": selb,
        })
        plans.append((b, toks, cnt[b, t0:t0 + per_batch] > 0))
    return win, in_maps, plans


def kernel(h_L, W_r, b_r, W1, b1, W2, b2, mask_indices, unmasked_indices,
           range_r):
    h_L = np.asarray(h_L, np.float32)
    mask_indices = np.asarray(mask_indices)
    unmasked_indices = np.asarray(unmasked_indices)
    assert h_L.shape == (B, S, D) and mask_indices.shape == (B, M)

    win, in_maps, plans = _host_prep(
        h_L, np.asarray(W_r, np.float32), np.asarray(b_r, np.float32),
        np.asarray(W1, np.float32), np.asarray(b1, np.float32),
        np.asarray(W2, np.float32), np.asarray(b2, np.float32),
        mask_indices, unmasked_indices, range_r)

    nc = get_nc(win)
    try:
        res = run_bass_kernel_spmd(nc, in_maps, core_ids=list(range(NCORES)))
    except Exception:
        # transient device faults (e.g. NRT_EXEC_UNIT_UNRECOVERABLE) happen
        # rarely under the axon tunnel; one retry clears them
        import time as _time
        _time.sleep(5)
        res = run_bass_kernel_spmd(nc, in_maps, core_ids=list(range(NCORES)))

    full = np.zeros((B, S, D), np.float32)
    for c in range(NCORES):
        b, toks, has = plans[c]
        o = np.asarray(res.results[c]["out"], np.float32)
        full[b, toks[has]] = o[has]
    return full
